# revision 1
# baseline (speedup 1.0000x reference)
"""Trainium2 Bass kernel for nn_Block_45552423141629 (pre-norm transformer
block with ELU linear attention), SPMD over 8 NeuronCores.

Sharding: sequence dimension N=8192 split into 8 shards of 1024 tokens; the
kv outer-product statistics ([B,H,64,64] + k_sum) are AllReduce'd across
cores once per batch. Everything else is fully local.

Self-contained: hardcodes shapes from the problem spec.
"""
import contextlib

import numpy as np
import ml_dtypes

import concourse.bass as bass
import concourse.mybir as mybir
import concourse.tile as tile
from concourse import bass_utils
from concourse.vector_clock import ScopedClock

# ---------------------------------------------------------------------------
# Workarounds: this walrus build accepts only ONE sync-wait per instruction.
# Split multi-waits onto unfusable NOPs on the same engine, and do the same
# for the TileContext tail drain.
# ---------------------------------------------------------------------------
_orig_lower = tile.TileContext._lower_ordered_insts


def _split_multi_waits(self, ordered):
    nc = self.nc
    for bb, insts in list(ordered.items()):
        new = []
        changed = False
        for inst in insts:
            si = inst.sync_info
            if si is not None and len(si.on_wait) > 1:
                waits = list(si.on_wait)
                for w in waits[:-1]:
                    nop = mybir.InstNoOp(
                        name=nc.get_next_instruction_name(),
                        ins=[],
                        outs=[],
                        bass_is_fusable=False,
                    )
                    nop.engine = inst.engine
                    nop.sync_info = mybir.SyncInfo(on_wait=[w], on_update=[])
                    new.append(nop)
                inst.sync_info = mybir.SyncInfo(
                    on_wait=[waits[-1]], on_update=list(si.on_update)
                )
                changed = True
            new.append(inst)
        if changed:
            ordered[bb] = new
    return _orig_lower(self, ordered)


if tile.TileContext._lower_ordered_insts is not _split_multi_waits:
    tile.TileContext._lower_ordered_insts = _split_multi_waits


def _patched_drain_and_barrier(self, tick_clock, wait_clock):
    nc = self.nc
    pre = nc.sync.nop(nofuse=True)
    wait_clock.add_sem_waits(pre.ins, ScopedClock({None: tick_clock.global_clock}))
    si = pre.ins.sync_info
    waits = list(si.on_wait) if si is not None else []
    if len(waits) > 1:
        pre.ins.sync_info = mybir.SyncInfo(
            on_wait=[waits[0]], on_update=list(si.on_update)
        )
        for w in waits[1:]:
            n2 = nc.sync.nop(nofuse=True)
            n2.ins.sync_info = mybir.SyncInfo(on_wait=[w], on_update=[])
    nc.sync.drain()
    nc.all_engine_barrier()
    popped = nc._tile_sem_poison_stack.pop()
    assert popped is self._sem_poison
    nc.clear_and_free_semaphores(list(self.sems.allocated().values()))
    nc.all_engine_barrier()


tile.TileContext._drain_and_barrier = _patched_drain_and_barrier

# ---------------------------------------------------------------------------

BF = ml_dtypes.bfloat16
F32 = mybir.dt.float32
BF16 = mybir.dt.bfloat16
AF = mybir.ActivationFunctionType
ALU = mybir.AluOpType

N_CORES = 8
B, N, D, H, HD, DFF = 4, 8192, 1024, 16, 64, 4096
NLOC = N // N_CORES        # 1024 tokens per core per batch
TC = NLOC // 128           # 8 token chunks per batch
DC = D // 128              # 8 dim chunks
GC = DFF // 128            # 32 ff chunks
NPAIR = H // 2             # 8 head pairs
EPS_LN = 1e-5
EPS_NORM = 1e-6

_nc_cache = {}


def _build(has_ckv: bool, has_c2: bool):
    key = (has_ckv, has_c2)
    if key in _nc_cache:
        return _nc_cache[key]

    nc = bass.Bass("TRN2", target_bir_lowering=False, debug=False,
                   num_devices=N_CORES)
    src = nc.dram_tensor("src", [B, NLOC, D], F32, kind="ExternalInput")
    # wq is packed [m, p, j*128+o] (stationary-tile layout, like fc1)
    wq = nc.dram_tensor("wq", [DC, 128, D], BF16, kind="ExternalInput")
    wk = nc.dram_tensor("wk", [DC, 128, D], BF16, kind="ExternalInput")
    wv = nc.dram_tensor("wv", [DC, 128, D], BF16, kind="ExternalInput")
    wo = nc.dram_tensor("wo", [DC, 128, D], BF16, kind="ExternalInput")
    fc1 = nc.dram_tensor("fc1", [GC, 128, D], BF16, kind="ExternalInput")
    fc2 = nc.dram_tensor("fc2", [GC, 128, D], BF16, kind="ExternalInput")
    c1 = nc.dram_tensor("c1", [128, GC], F32, kind="ExternalInput")
    cq = nc.dram_tensor("cq", [128, DC], F32, kind="ExternalInput")
    if has_ckv:
        ckv = nc.dram_tensor("ckv", [2, D], F32, kind="ExternalInput")
    if has_c2:
        c2 = nc.dram_tensor("c2", [D], F32, kind="ExternalInput")
    out = nc.dram_tensor("out", [B, NLOC, D], F32, kind="ExternalOutput")

    with tile.TileContext(nc) as tc:
        ctx = contextlib.ExitStack()
        with ctx:
            p_w = ctx.enter_context(tc.tile_pool(name="p_w", bufs=16))
            p_fc1 = ctx.enter_context(tc.tile_pool(name="p_fc1", bufs=3))
            p_fc2 = ctx.enter_context(tc.tile_pool(name="p_fc2", bufs=3))
            p_x = ctx.enter_context(tc.tile_pool(name="p_x", bufs=2))
            p_hT = ctx.enter_context(tc.tile_pool(name="p_hT", bufs=DC))
            p_qT = ctx.enter_context(tc.tile_pool(name="p_qT", bufs=DC))
            p_k = ctx.enter_context(tc.tile_pool(name="p_k", bufs=3))
            p_v = ctx.enter_context(tc.tile_pool(name="p_v", bufs=3))
            p_aT = ctx.enter_context(tc.tile_pool(name="p_aT", bufs=NPAIR))
            p_s2 = ctx.enter_context(tc.tile_pool(name="p_s2", bufs=1))
            p_h2T = ctx.enter_context(tc.tile_pool(name="p_h2T", bufs=DC))
            p_gt = ctx.enter_context(tc.tile_pool(name="p_gt", bufs=GC))
            p_rnb = ctx.enter_context(tc.tile_pool(name="p_rnb", bufs=2))
            p_tmp = ctx.enter_context(tc.tile_pool(name="p_tmp", bufs=2))
            p_ae = ctx.enter_context(tc.tile_pool(name="p_ae", bufs=4))
            p_sm = ctx.enter_context(tc.tile_pool(name="p_sm", bufs=1))
            p_st = ctx.enter_context(tc.tile_pool(name="p_st", bufs=3))
            p_one = ctx.enter_context(tc.tile_pool(name="p_one", bufs=1))
            p_ob = ctx.enter_context(tc.tile_pool(name="p_ob", bufs=2))
            ps_mm = ctx.enter_context(
                tc.tile_pool(name="ps_mm", bufs=6, space="PSUM"))
            ps_kv = ctx.enter_context(
                tc.tile_pool(name="ps_kv", bufs=1, space="PSUM"))
            dram = ctx.enter_context(
                tc.tile_pool(name="dramp", bufs=4, space="DRAM"))
            dram_s = ctx.enter_context(
                tc.tile_pool(name="dramps", bufs=4, space="DRAM"))
            dram_s2 = ctx.enter_context(
                tc.tile_pool(name="drams2", bufs=2 * TC, space="DRAM"))

            # --- constants ---
            c1_sb = p_one.tile([128, GC], F32, tag="c1", name="c1")
            nc.sync.dma_start(out=c1_sb, in_=c1.ap())
            cq_sb = p_one.tile([128, DC], F32, tag="cq", name="cq")
            nc.sync.dma_start(out=cq_sb, in_=cq.ap())
            eps_sb = p_one.tile([128, 1], F32, tag="eps", name="eps")
            nc.vector.memset(eps_sb, EPS_LN)
            if has_ckv:
                ck_b = p_one.tile([128, D], F32, tag="ckb", name="ckb")
                cv_b = p_one.tile([128, D], F32, tag="cvb", name="cvb")
                ckap = ckv.ap()
                for idx, t in ((0, ck_b), (1, cv_b)):
                    nc.sync.dma_start(
                        out=t,
                        in_=bass.AP(tensor=ckap.tensor, offset=idx * D,
                                    ap=[[0, 128], [1, D]]))
            if has_c2:
                c2_b = p_one.tile([128, D], F32, tag="c2b", name="c2b")
                c2ap = c2.ap()
                nc.sync.dma_start(
                    out=c2_b,
                    in_=bass.AP(tensor=c2ap.tensor, offset=0,
                                ap=[[0, 128], [1, D]]))

            def ln_stats(xt):
                """mean/rstd of [128, D] fp32 tile -> (mv, rstd)."""
                st = p_st.tile([128, 2, 6], F32, tag="st", name="st")
                xr = xt.rearrange("p (s f) -> p s f", s=2)
                for s in range(2):
                    nc.vector.bn_stats(out=st[:, s, :], in_=xr[:, s, :])
                mv = p_st.tile([128, 2], F32, tag="mv", name="mv")
                nc.vector.bn_aggr(out=mv, in_=st)
                rstd = p_st.tile([128, 1], F32, tag="rstd", name="rstd")
                nc.scalar.activation(out=rstd, in_=mv[:, 1:2], func=AF.Sqrt,
                                     bias=eps_sb, scale=1.0)
                nc.vector.reciprocal(out=rstd, in_=rstd)
                return mv, rstd

            for b in range(B):
                # ---------------- Phase A: LN1 + transpose ----------------
                hT = [p_hT.tile([128, NLOC], BF16, tag="hT", name="hT")
                      for _ in range(DC)]
                for i in range(TC):
                    xt = p_x.tile([128, D], F32, tag="x", name="x")
                    nc.sync.dma_start(
                        out=xt, in_=src.ap()[b, i * 128:(i + 1) * 128, :])
                    mv, rstd = ln_stats(xt)
                    h = p_tmp.tile([128, D], BF16, tag="h", name="h")
                    nc.vector.tensor_scalar(
                        out=h, in0=xt, scalar1=mv[:, 0:1], scalar2=rstd,
                        op0=ALU.subtract, op1=ALU.mult)
                    for j in range(DC):
                        nc.sync.dma_start_transpose(
                            hT[j][:, i * 128:(i + 1) * 128],
                            h[:, j * 128:(j + 1) * 128])

                # ------- Phase B: k/v projections + incremental kv --------
                wk_sb = [p_w.tile([128, D], BF16, tag="w", name="wk_sb")
                         for _ in range(DC)]
                wv_sb = [p_w.tile([128, D], BF16, tag="w", name="wv_sb")
                         for _ in range(DC)]
                for j in range(DC):
                    nc.sync.dma_start(out=wk_sb[j], in_=wk.ap()[j])
                for j in range(DC):
                    nc.sync.dma_start(out=wv_sb[j], in_=wv.ap()[j])

                # kv+ksum accumulator: [128, pair, 128] fp32 = 2 PSUM banks;
                # per (pair, head): [64, 65] block at 512B-aligned offsets.
                pkv = ps_kv.tile([128, NPAIR, 128], F32, tag="kv", name="pkv")
                for i in range(TC):
                    k_t = p_k.tile([128, D], BF16, tag="k", name="k_t")
                    v_t = p_v.tile([128, H, HD + 1], BF16, tag="v", name="v_t")
                    nc.vector.memset(v_t[:, :, HD:HD + 1], 1.0)
                    for ncol in range(2):
                        csl = slice(ncol * 512, (ncol + 1) * 512)
                        # k
                        pk = ps_mm.tile([128, 512], F32, tag="mm", name="pk")
                        for j in range(DC):
                            nc.tensor.matmul(
                                pk, hT[j][:, i * 128:(i + 1) * 128],
                                wk_sb[j][:, csl],
                                start=(j == 0), stop=(j == DC - 1))
                        if has_ckv:
                            kb = p_tmp.tile([128, 512], F32, tag="mn",
                                            name="kb")
                            nc.vector.scalar_tensor_tensor(
                                out=kb, in0=pk, scalar=0.0, in1=ck_b[:, csl],
                                op0=ALU.add, op1=ALU.add)
                            ksrc = kb
                        else:
                            ksrc = pk
                        rl = p_ae.tile([128, 512], BF16, tag="ae", name="rl")
                        nc.scalar.activation(out=rl, in_=ksrc, func=AF.Relu)
                        mn = p_tmp.tile([128, 512], F32, tag="mn", name="mn")
                        nc.vector.tensor_scalar_min(out=mn, in0=ksrc,
                                                    scalar1=0.0)
                        ex = p_ae.tile([128, 512], BF16, tag="ae", name="ex")
                        nc.scalar.activation(out=ex, in_=mn, func=AF.Exp)
                        nc.vector.tensor_add(out=k_t[:, csl], in0=ex, in1=rl)
                        # v
                        pv = ps_mm.tile([128, 512], F32, tag="mm", name="pv")
                        for j in range(DC):
                            nc.tensor.matmul(
                                pv, hT[j][:, i * 128:(i + 1) * 128],
                                wv_sb[j][:, csl],
                                start=(j == 0), stop=(j == DC - 1))
                        vdst = v_t[:, ncol * 8:(ncol + 1) * 8, 0:HD]
                        pvr = pv.rearrange("p (h e) -> p h e", e=HD)
                        if has_ckv:
                            cvr = cv_b[:, csl].rearrange(
                                "p (h e) -> p h e", e=HD)
                            nc.vector.scalar_tensor_tensor(
                                out=vdst, in0=pvr, scalar=0.0, in1=cvr,
                                op0=ALU.add, op1=ALU.add)
                        else:
                            nc.vector.tensor_copy(out=vdst, in_=pvr)
                    # accumulate kv for all head pairs from this chunk
                    for hp in range(NPAIR):
                        hA, hB = 2 * hp, 2 * hp + 1
                        nc.tensor.matmul(
                            pkv[0:64, hp, 0:HD + 1],
                            k_t[:, hA * HD:(hA + 1) * HD],
                            v_t[:, hA, :],
                            start=(i == 0), stop=(i == TC - 1),
                            tile_position=(0, 0), skip_group_check=True)
                        nc.tensor.matmul(
                            pkv[64:128, hp, 0:HD + 1],
                            k_t[:, hB * HD:(hB + 1) * HD],
                            v_t[:, hB, :],
                            start=(i == 0), stop=(i == TC - 1),
                            tile_position=(0, 64), skip_group_check=True)

                kv_sb = p_sm.tile([128, NPAIR, HD + 1], F32, tag="kvsb",
                                  name="kvsb")
                nc.vector.tensor_copy(out=kv_sb, in_=pkv[:, :, 0:HD + 1])
                kv_in = dram.tile([128, NPAIR, HD + 1], F32, tag="kvin",
                                  name="kvin")
                kv_out = dram_s.tile([128, NPAIR, HD + 1], F32, tag="kvout",
                                     name="kvout", addr_space="Shared")
                nc.sync.dma_start(out=kv_in, in_=kv_sb)
                nc.gpsimd.collective_compute(
                    "AllReduce", ALU.add,
                    replica_groups=[list(range(N_CORES))],
                    ins=[kv_in.opt()], outs=[kv_out.opt()])

                # ---------------- Phase B3: q projection (overlaps AR) ----
                qT = [p_qT.tile([128, NLOC], BF16, tag="qT", name="qT")
                      for _ in range(DC)]
                for m in range(DC):
                    wqm = p_fc1.tile([128, DC, 128], BF16, tag="f1",
                                     name="wqm")
                    nc.sync.dma_start(
                        out=wqm,
                        in_=wq.ap()[m].rearrange("p (j e) -> p j e", j=DC))
                    for ncol in range(2):
                        csl = slice(ncol * 512, (ncol + 1) * 512)
                        pq = ps_mm.tile([128, 512], F32, tag="mm", name="pq")
                        for j in range(DC):
                            nc.tensor.matmul(
                                pq, wqm[:, j, :], hT[j][:, csl],
                                start=(j == 0), stop=(j == DC - 1))
                        rl = p_ae.tile([128, 512], BF16, tag="ae", name="rlq")
                        nc.scalar.activation(out=rl, in_=pq, func=AF.Relu,
                                             bias=cq_sb[:, m:m + 1], scale=1.0)
                        mn = p_tmp.tile([128, 512], F32, tag="mn", name="mnq")
                        nc.vector.tensor_scalar(
                            out=mn, in0=pq, scalar1=cq_sb[:, m:m + 1],
                            scalar2=0.0, op0=ALU.add, op1=ALU.min)
                        ex = p_ae.tile([128, 512], BF16, tag="ae", name="exq")
                        nc.scalar.activation(out=ex, in_=mn, func=AF.Exp)
                        nc.vector.tensor_add(out=qT[m][:, csl], in0=ex, in1=rl)

                # ---------------- Phase D: attention ---------------------
                kv_red = p_sm.tile([128, NPAIR, HD + 1], F32, tag="kvred",
                                   name="kvred")
                nc.sync.dma_start(out=kv_red, in_=kv_out)
                kvb = p_sm.tile([128, NPAIR, HD + 1], BF16, tag="kvb",
                                name="kvb")
                nc.vector.tensor_copy(out=kvb, in_=kv_red)

                # normalizers: accumulate block-diag ksum matmuls
                pn = [ps_mm.tile([16, 512], F32, tag="mm", name="pn")
                      for _ in range(2)]
                for hp in range(NPAIR):
                    ks16 = p_sm.tile([128, 16], BF16, tag="ks16", name="ks16",
                                     bufs=NPAIR)
                    nc.vector.memset(ks16, 0.0)
                    nc.vector.tensor_copy(
                        out=ks16[0:64, 2 * hp:2 * hp + 1],
                        in_=kvb[0:64, hp, HD:HD + 1])
                    nc.vector.tensor_copy(
                        out=ks16[64:128, 2 * hp + 1:2 * hp + 2],
                        in_=kvb[64:128, hp, HD:HD + 1])
                    for ncol in range(2):
                        nc.tensor.matmul(
                            pn[ncol], ks16,
                            qT[hp][:, ncol * 512:(ncol + 1) * 512],
                            start=(hp == 0), stop=(hp == NPAIR - 1),
                            skip_group_check=True)
                n16 = p_sm.tile([16, NLOC], F32, tag="n16", name="n16")
                for ncol in range(2):
                    nc.vector.tensor_scalar_add(
                        out=n16[:, ncol * 512:(ncol + 1) * 512],
                        in0=pn[ncol], scalar1=EPS_NORM)
                rn16 = p_sm.tile([16, NLOC], BF16, tag="rn16", name="rn16")
                with nc.allow_low_precision(reason="rn broadcast in bf16"):
                    nc.vector.reciprocal(out=rn16, in_=n16)
                rn_d = dram.tile([16, NLOC], BF16, tag="rnd", name="rnd")
                nc.sync.dma_start(out=rn_d, in_=rn16)

                aT = [p_aT.tile([128, NLOC], BF16, tag="aT", name="aT")
                      for _ in range(NPAIR)]
                for hp in range(NPAIR):
                    rnb = p_rnb.tile([128, NLOC], BF16, tag="rnb", name="rnb")
                    rnap = rn_d.opt()
                    for hh in range(2):
                        nc.sync.dma_start(
                            out=rnb[hh * 64:(hh + 1) * 64, :],
                            in_=bass.AP(
                                tensor=rnap.tensor,
                                offset=rnap.offset + (2 * hp + hh) * NLOC,
                                ap=[[0, 64], [1, NLOC]]))
                    for ncol in range(2):
                        csl = slice(ncol * 512, (ncol + 1) * 512)
                        po = ps_mm.tile([128, 512], F32, tag="mm", name="po")
                        nc.tensor.matmul(
                            po[0:64, :], kvb[0:64, hp, 0:HD],
                            qT[hp][0:64, csl],
                            start=True, stop=True, tile_position=(0, 0))
                        nc.tensor.matmul(
                            po[64:128, :], kvb[64:128, hp, 0:HD],
                            qT[hp][64:128, csl],
                            start=True, stop=True, tile_position=(64, 64))
                        nc.vector.tensor_mul(
                            out=aT[hp][:, csl], in0=po, in1=rnb[:, csl])

                # ---------------- Phase E: wo + residual + LN2 -----------
                wo_sb = [p_w.tile([128, D], BF16, tag="w", name="wo_sb")
                         for _ in range(DC)]
                for j in range(DC):
                    nc.sync.dma_start(out=wo_sb[j], in_=wo.ap()[j])
                h2T = [p_h2T.tile([128, NLOC], BF16, tag="h2T", name="h2T")
                       for _ in range(DC)]
                s2d = [dram_s2.tile([128, D], F32, tag="s2d", name="s2d")
                       for _ in range(TC)]
                for i in range(TC):
                    x2 = p_x.tile([128, D], F32, tag="x", name="x2")
                    nc.sync.dma_start(
                        out=x2, in_=src.ap()[b, i * 128:(i + 1) * 128, :])
                    s2 = p_s2.tile([128, D], F32, tag="s2", name="s2")
                    for ncol in range(2):
                        csl = slice(ncol * 512, (ncol + 1) * 512)
                        py = ps_mm.tile([128, 512], F32, tag="mm", name="py")
                        for hp in range(NPAIR):
                            nc.tensor.matmul(
                                py, aT[hp][:, i * 128:(i + 1) * 128],
                                wo_sb[hp][:, csl],
                                start=(hp == 0), stop=(hp == NPAIR - 1))
                        nc.vector.tensor_add(out=s2[:, csl], in0=py,
                                             in1=x2[:, csl])
                    nc.sync.dma_start(out=s2d[i], in_=s2)
                    mv2, rstd2 = ln_stats(s2)
                    h2 = p_tmp.tile([128, D], BF16, tag="h", name="h2")
                    nc.vector.tensor_scalar(
                        out=h2, in0=s2, scalar1=mv2[:, 0:1], scalar2=rstd2,
                        op0=ALU.subtract, op1=ALU.mult)
                    for j in range(DC):
                        nc.sync.dma_start_transpose(
                            h2T[j][:, i * 128:(i + 1) * 128],
                            h2[:, j * 128:(j + 1) * 128])

                # ---------------- Phase G/H: MLP, per t-half -------------
                for half in range(2):
                    tsl = slice(half * 512, (half + 1) * 512)
                    gt = [p_gt.tile([128, 512], BF16, tag="gt", name="gt")
                          for _ in range(GC)]
                    for m in range(GC):
                        f1 = p_fc1.tile([128, DC, 128], BF16, tag="f1",
                                        name="f1")
                        nc.sync.dma_start(
                            out=f1,
                            in_=fc1.ap()[m].rearrange("p (j e) -> p j e",
                                                      j=DC))
                        pu = ps_mm.tile([128, 512], F32, tag="mm", name="pu")
                        for j in range(DC):
                            nc.tensor.matmul(
                                pu, f1[:, j, :], h2T[j][:, tsl],
                                start=(j == 0), stop=(j == DC - 1))
                        nc.scalar.activation(out=gt[m], in_=pu, func=AF.Gelu,
                                             bias=c1_sb[:, m:m + 1], scale=1.0)
                    for ncol in range(2):
                        csl = slice(ncol * 512, (ncol + 1) * 512)
                        py2 = [ps_mm.tile([128, 512], F32, tag="mm",
                                          name="py2") for _ in range(4)]
                        for m in range(GC):
                            f2 = p_fc2.tile([128, 512], BF16, tag="f2",
                                            name="f2")
                            nc.sync.dma_start(out=f2, in_=fc2.ap()[m][:, csl])
                            for ii in range(4):
                                nc.tensor.matmul(
                                    py2[ii],
                                    gt[m][:, ii * 128:(ii + 1) * 128], f2,
                                    start=(m == 0), stop=(m == GC - 1))
                        for ii in range(4):
                            i = half * 4 + ii
                            s2c = p_ob.tile([128, 512], F32, tag="s2c",
                                            name="s2c")
                            nc.sync.dma_start(out=s2c, in_=s2d[i][:, csl])
                            ot = p_ob.tile([128, 512], F32, tag="ot",
                                           name="ot")
                            if has_c2:
                                nc.vector.scalar_tensor_tensor(
                                    out=ot, in0=py2[ii], scalar=0.0,
                                    in1=c2_b[:, csl], op0=ALU.add, op1=ALU.add)
                                nc.vector.tensor_add(out=ot, in0=ot, in1=s2c)
                            else:
                                nc.vector.tensor_add(out=ot, in0=py2[ii],
                                                     in1=s2c)
                            nc.sync.dma_start(
                                out=out.ap()[b, i * 128:(i + 1) * 128, csl],
                                in_=ot)

    _nc_cache[key] = nc
    return nc


def kernel(**inputs) -> np.ndarray:
    src = np.ascontiguousarray(np.asarray(inputs["src"], dtype=np.float32))
    ln1_w = np.asarray(inputs["ln1_w"], np.float32)
    ln1_b = np.asarray(inputs["ln1_b"], np.float32)
    wq = np.asarray(inputs["wq"], np.float32)
    wk = np.asarray(inputs["wk"], np.float32)
    wv = np.asarray(inputs["wv"], np.float32)
    wo = np.asarray(inputs["wo"], np.float32)
    ln2_w = np.asarray(inputs["ln2_w"], np.float32)
    ln2_b = np.asarray(inputs["ln2_b"], np.float32)
    fc1_w = np.asarray(inputs["fc1_w"], np.float32)
    fc1_b = np.asarray(inputs["fc1_b"], np.float32)
    fc2_w = np.asarray(inputs["fc2_w"], np.float32)
    fc2_b = np.asarray(inputs["fc2_b"], np.float32)

    # host-side folds (exact, input-value independent transformations)
    wqf = ((ln1_w[:, None] * wq).astype(BF)
           .reshape(DC, 128, DC, 128).transpose(2, 1, 0, 3)
           .reshape(DC, 128, D).copy())
    wkf = (ln1_w[:, None] * wk).astype(BF).reshape(DC, 128, D)
    wvf = (ln1_w[:, None] * wv).astype(BF).reshape(DC, 128, D)
    wof = wo.astype(BF).reshape(DC, 128, D)
    fc1f = ((ln2_w[:, None] * fc1_w).astype(BF)
            .reshape(DC, 128, GC, 128).transpose(2, 1, 0, 3)
            .reshape(GC, 128, D).copy())
    fc2f = fc2_w.astype(BF).reshape(GC, 128, D)
    cq_v = ln1_b @ wq
    ck_v = ln1_b @ wk
    cv_v = ln1_b @ wv
    c1_v = ln2_b @ fc1_w + fc1_b
    has_ckv = bool(np.any(ck_v) or np.any(cv_v))
    has_c2 = bool(np.any(fc2_b))

    base = {
        "wq": wqf, "wk": wkf, "wv": wvf, "wo": wof,
        "fc1": fc1f, "fc2": fc2f,
        "c1": np.ascontiguousarray(c1_v.reshape(GC, 128).T.astype(np.float32)),
        "cq": np.ascontiguousarray(cq_v.reshape(DC, 128).T.astype(np.float32)),
    }
    if has_ckv:
        base["ckv"] = np.stack([ck_v, cv_v]).astype(np.float32)
    if has_c2:
        base["c2"] = fc2_b.astype(np.float32)

    nc = _build(has_ckv, has_c2)
    in_maps = []
    for c in range(N_CORES):
        m = dict(base)
        m["src"] = np.ascontiguousarray(src[:, c * NLOC:(c + 1) * NLOC, :])
        in_maps.append(m)
    res = bass_utils.run_bass_kernel_spmd(
        nc, in_maps, core_ids=list(range(N_CORES)))
    return np.concatenate(
        [res.results[c]["out"] for c in range(N_CORES)], axis=1)



# revision 2
# speedup vs baseline: 1.0034x; 1.0034x over previous
"""Trainium2 Bass kernel for nn_Block_45552423141629 (pre-norm transformer
block with ELU linear attention), SPMD over 8 NeuronCores.

v2: fp8-e4m3 DoubleRow matmuls for the whole attention path (qkv/kv/app/wo),
bf16 MLP (optionally fp8 fc1), software-pipelined batches so the MLP of
batch b-1 covers the kv AllReduce latency of batch b.

Sharding: sequence dimension N=8192 split into 8 shards of 1024 tokens; the
kv outer-product statistics ([B,H,64,65]) are AllReduce'd across cores once
per batch. Everything else is fully local.

Self-contained: hardcodes shapes from the problem spec.
"""
import contextlib
import math

import numpy as np
import ml_dtypes

import concourse.bass as bass
import concourse.mybir as mybir
import concourse.tile as tile
from concourse import bass_utils
from concourse.vector_clock import ScopedClock

# ---------------------------------------------------------------------------
# Workarounds: this walrus build accepts only ONE sync-wait per instruction.
# Split multi-waits onto unfusable NOPs on the same engine, and do the same
# for the TileContext tail drain.
# ---------------------------------------------------------------------------
_orig_lower = tile.TileContext._lower_ordered_insts


def _split_multi_waits(self, ordered):
    nc = self.nc
    for bb, insts in list(ordered.items()):
        new = []
        changed = False
        for inst in insts:
            si = inst.sync_info
            if si is not None and len(si.on_wait) > 1:
                waits = list(si.on_wait)
                for w in waits[:-1]:
                    nop = mybir.InstNoOp(
                        name=nc.get_next_instruction_name(),
                        ins=[],
                        outs=[],
                        bass_is_fusable=False,
                    )
                    nop.engine = inst.engine
                    nop.sync_info = mybir.SyncInfo(on_wait=[w], on_update=[])
                    new.append(nop)
                inst.sync_info = mybir.SyncInfo(
                    on_wait=[waits[-1]], on_update=list(si.on_update)
                )
                changed = True
            new.append(inst)
        if changed:
            ordered[bb] = new
    return _orig_lower(self, ordered)


if tile.TileContext._lower_ordered_insts is not _split_multi_waits:
    tile.TileContext._lower_ordered_insts = _split_multi_waits


def _patched_drain_and_barrier(self, tick_clock, wait_clock):
    nc = self.nc
    pre = nc.sync.nop(nofuse=True)
    wait_clock.add_sem_waits(pre.ins, ScopedClock({None: tick_clock.global_clock}))
    si = pre.ins.sync_info
    waits = list(si.on_wait) if si is not None else []
    if len(waits) > 1:
        pre.ins.sync_info = mybir.SyncInfo(
            on_wait=[waits[0]], on_update=list(si.on_update)
        )
        for w in waits[1:]:
            n2 = nc.sync.nop(nofuse=True)
            n2.ins.sync_info = mybir.SyncInfo(on_wait=[w], on_update=[])
    nc.sync.drain()
    nc.all_engine_barrier()
    popped = nc._tile_sem_poison_stack.pop()
    assert popped == self._sem_poison
    nc.clear_and_free_semaphores(list(self.sems.allocated().values()))
    nc.all_engine_barrier()


tile.TileContext._drain_and_barrier = _patched_drain_and_barrier

# ---------------------------------------------------------------------------

BF = ml_dtypes.bfloat16
E4 = ml_dtypes.float8_e4m3
F32 = mybir.dt.float32
BF16 = mybir.dt.bfloat16
FP8 = mybir.dt.float8e4
AF = mybir.ActivationFunctionType
ALU = mybir.AluOpType
DRM = mybir.MatmulPerfMode.DoubleRow

N_CORES = 8
B, N, D, H, HD, DFF = 4, 8192, 1024, 16, 64, 4096
NLOC = N // N_CORES        # 1024 tokens per core per batch
TC = NLOC // 128           # 8 token chunks per batch
DC = D // 128              # 8 dim chunks
QD = DC // 2               # 4 dim chunk-pairs (DoubleRow planes)
GC = DFF // 128            # 32 ff chunks
NPAIR = H // 2             # 8 head pairs
PP = NPAIR // 2            # 4 head-pair pairs (wo DoubleRow planes)
EPS_LN = 1e-5
EPS_NORM = 1e-6
S_H = 16.0                 # LN output fp8 scale (LN output is ~N(0,1))

FC1_FP8 = True             # fc1 in fp8 DoubleRow (rel err 1.66e-2 < 2e-2 gate)

# cst column indices
C_SK_RL, C_SK_DQ, C_SK_LN, C_SV_M, C_KV_M, C_KS_M, C_N_M, C_RN_M, \
    C_WO_DQ, C_SQ_LN, C_SK, C_SQ = range(12)
NCST = 12

_nc_cache = {}


def _build(has_ckv: bool, has_c2: bool, fc1_fp8: bool):
    key = (has_ckv, has_c2, fc1_fp8)
    if key in _nc_cache:
        return _nc_cache[key]

    nc = bass.Bass("TRN2", target_bir_lowering=False, debug=False,
                   num_devices=N_CORES)
    src = nc.dram_tensor("src", [B, NLOC, D], F32, kind="ExternalInput")
    # fp8 attention weights, plane-blocked: w8[c, p, q, f] = W[128*(2c+q)+p, f]
    wk8 = nc.dram_tensor("wk8", [QD, 128, 2, D], FP8, kind="ExternalInput")
    wv8 = nc.dram_tensor("wv8", [QD, 128, 2, D], FP8, kind="ExternalInput")
    wq8 = nc.dram_tensor("wq8", [QD, 128, 2, D], FP8, kind="ExternalInput")
    wo8 = nc.dram_tensor("wo8", [PP, 128, 2, D], FP8, kind="ExternalInput")
    if fc1_fp8:
        fc18 = nc.dram_tensor("fc18", [QD, 128, 2, DFF], FP8,
                              kind="ExternalInput")
    else:
        # fc1[m][p, 128j+e] = fc1_w[128j+p, 128m+e]  (stationary layout)
        fc1 = nc.dram_tensor("fc1", [GC, 128, D], BF16, kind="ExternalInput")
    fc2 = nc.dram_tensor("fc2", [GC, 128, D], BF16, kind="ExternalInput")
    cst = nc.dram_tensor("cst", [128, NCST], F32, kind="ExternalInput")
    ident = nc.dram_tensor("ident", [128, 128], BF16, kind="ExternalInput")
    cdqs = nc.dram_tensor("cdqs", [128, DC], F32, kind="ExternalInput")
    cdq = nc.dram_tensor("cdq", [128, DC], F32, kind="ExternalInput")
    c1b = nc.dram_tensor("c1b", [128, GC], F32, kind="ExternalInput")
    if fc1_fp8:
        cdg = nc.dram_tensor("cdg", [128, GC], F32, kind="ExternalInput")
    if has_ckv:
        # k bias (raw), v bias (pre-scaled by S_V), q bias raw + scaled
        ckv = nc.dram_tensor("ckv", [2, D], F32, kind="ExternalInput")
        cqb2 = nc.dram_tensor("cqb2", [128, 2 * DC], F32,
                              kind="ExternalInput")
    if has_c2:
        c2 = nc.dram_tensor("c2", [D], F32, kind="ExternalInput")
    out = nc.dram_tensor("out", [B, NLOC, D], F32, kind="ExternalOutput")

    with tile.TileContext(nc) as tc:
        ctx = contextlib.ExitStack()
        with ctx:
            p_w8 = ctx.enter_context(tc.tile_pool(name="p_w8", bufs=16))
            p_one = ctx.enter_context(tc.tile_pool(name="p_one", bufs=1))
            p_x = ctx.enter_context(tc.tile_pool(name="p_x", bufs=2))
            p_h = ctx.enter_context(tc.tile_pool(name="p_h", bufs=3))
            p_hT8 = ctx.enter_context(tc.tile_pool(name="p_hT8", bufs=1))
            p_k8 = ctx.enter_context(tc.tile_pool(name="p_k8", bufs=1))
            p_v8 = ctx.enter_context(tc.tile_pool(name="p_v8", bufs=1))
            p_qT8 = ctx.enter_context(tc.tile_pool(name="p_qT8", bufs=1))
            p_aT8 = ctx.enter_context(tc.tile_pool(name="p_aT8", bufs=1))
            if fc1_fp8:
                p_h2T8 = ctx.enter_context(
                    tc.tile_pool(name="p_h2T8", bufs=1))
                p_f18 = ctx.enter_context(tc.tile_pool(name="p_f18", bufs=4))
            else:
                p_h2T = ctx.enter_context(
                    tc.tile_pool(name="p_h2T", bufs=1))
                p_f1 = ctx.enter_context(tc.tile_pool(name="p_f1", bufs=4))
            p_ae = ctx.enter_context(tc.tile_pool(name="p_ae", bufs=4))
            p_tmp = ctx.enter_context(tc.tile_pool(name="p_tmp", bufs=1))
            p_s2 = ctx.enter_context(tc.tile_pool(name="p_s2", bufs=2))
            p_sm = ctx.enter_context(tc.tile_pool(name="p_sm", bufs=1))
            p_st = ctx.enter_context(tc.tile_pool(name="p_st", bufs=4))
            p_gt = ctx.enter_context(tc.tile_pool(name="p_gt", bufs=GC))
            p_f2 = ctx.enter_context(tc.tile_pool(name="p_f2", bufs=4))
            p_ob = ctx.enter_context(tc.tile_pool(name="p_ob", bufs=2))
            p_rnb = ctx.enter_context(tc.tile_pool(name="p_rnb", bufs=2))
            ps_mm = ctx.enter_context(
                tc.tile_pool(name="ps_mm", bufs=5, space="PSUM"))
            ps_kv = ctx.enter_context(
                tc.tile_pool(name="ps_kv", bufs=1, space="PSUM"))
            dram = ctx.enter_context(
                tc.tile_pool(name="dramp", bufs=4, space="DRAM"))
            dram_s = ctx.enter_context(
                tc.tile_pool(name="dramps", bufs=4, space="DRAM"))
            dram_s2 = ctx.enter_context(
                tc.tile_pool(name="drams2", bufs=2 * TC, space="DRAM"))

            # ---------------- constants + weights -------------------------
            cst_sb = p_one.tile([128, NCST], F32, tag="cst", name="cst")
            nc.sync.dma_start(out=cst_sb, in_=cst.ap())
            cdqs_sb = p_one.tile([128, DC], F32, tag="cdqs", name="cdqs")
            nc.sync.dma_start(out=cdqs_sb, in_=cdqs.ap())
            cdq_sb = p_one.tile([128, DC], F32, tag="cdq", name="cdq")
            nc.sync.dma_start(out=cdq_sb, in_=cdq.ap())
            c1b_sb = p_one.tile([128, GC], F32, tag="c1b", name="c1b")
            nc.sync.dma_start(out=c1b_sb, in_=c1b.ap())
            if fc1_fp8:
                cdg_sb = p_one.tile([128, GC], F32, tag="cdg", name="cdg")
                nc.sync.dma_start(out=cdg_sb, in_=cdg.ap())
            id_sb = p_one.tile([128, 128], BF16, tag="ident", name="ident")
            nc.sync.dma_start(out=id_sb, in_=ident.ap())
            eps_sb = p_one.tile([128, 1], F32, tag="eps", name="eps")
            nc.vector.memset(eps_sb, EPS_LN)
            if has_ckv:
                ck_b = p_one.tile([128, D], F32, tag="ckb", name="ckb")
                cv_b = p_one.tile([128, D], F32, tag="cvb", name="cvb")
                ckap = ckv.ap()
                for idx, t in ((0, ck_b), (1, cv_b)):
                    nc.sync.dma_start(
                        out=t,
                        in_=bass.AP(tensor=ckap.tensor, offset=idx * D,
                                    ap=[[0, 128], [1, D]]))
                cqb_sb = p_one.tile([128, 2 * DC], F32, tag="cqb",
                                    name="cqb")
                nc.sync.dma_start(out=cqb_sb, in_=cqb2.ap())
            if has_c2:
                c2_b = p_one.tile([128, D], F32, tag="c2b", name="c2b")
                c2ap = c2.ap()
                nc.sync.dma_start(
                    out=c2_b,
                    in_=bass.AP(tensor=c2ap.tensor, offset=0,
                                ap=[[0, 128], [1, D]]))

            wk8s = [p_w8.tile([128, 2, D], FP8, tag="w8", name="wk8s")
                    for _ in range(QD)]
            wv8s = [p_w8.tile([128, 2, D], FP8, tag="w8", name="wv8s")
                    for _ in range(QD)]
            wq8s = [p_w8.tile([128, 2, D], FP8, tag="w8", name="wq8s")
                    for _ in range(QD)]
            wo8s = [p_w8.tile([128, 2, D], FP8, tag="w8", name="wo8s")
                    for _ in range(QD)]
            for c in range(QD):
                nc.sync.dma_start(out=wk8s[c], in_=wk8.ap()[c])
                nc.sync.dma_start(out=wv8s[c], in_=wv8.ap()[c])
                nc.sync.dma_start(out=wq8s[c], in_=wq8.ap()[c])
                nc.sync.dma_start(out=wo8s[c], in_=wo8.ap()[c])
            if fc1_fp8:
                f18s = [p_f18.tile([128, 2, DFF], FP8, tag="f18",
                                   name="f18s") for _ in range(QD)]
                for c in range(QD):
                    nc.sync.dma_start(out=f18s[c], in_=fc18.ap()[c])

            def ln_stats(xt):
                """mean/rstd of [128, D] fp32 tile -> (mv, rstd)."""
                st = p_st.tile([128, 2, 6], F32, tag="st", name="st")
                xr = xt.rearrange("p (s f) -> p s f", s=2)
                for s in range(2):
                    nc.vector.bn_stats(out=st[:, s, :], in_=xr[:, s, :])
                mv = p_st.tile([128, 2], F32, tag="mv", name="mv")
                nc.vector.bn_aggr(out=mv, in_=st)
                rstd = p_st.tile([128, 1], F32, tag="rstd", name="rstd")
                nc.scalar.activation(out=rstd, in_=mv[:, 1:2], func=AF.Sqrt,
                                     bias=eps_sb, scale=1.0)
                nc.vector.reciprocal(out=rstd, in_=rstd)
                return mv, rstd

            def ln_norm(xt):
                """LayerNorm a [128, D] fp32 chunk to a bf16 tile."""
                mv, rstd = ln_stats(xt)
                hb = p_h.tile([128, D], BF16, tag="h", name="hb")
                nc.vector.tensor_scalar(
                    out=hb, in0=xt, scalar1=mv[:, 0:1], scalar2=rstd,
                    op0=ALU.subtract, op1=ALU.mult)
                return hb

            state = {}
            sinks = []

            def pe_transpose(hb, sink):
                """PE-transpose a [128, D] bf16 chunk; sink(tr) consumes the
                [128, DC, 128] transposed psum tile."""
                tr = ps_mm.tile([128, DC, 128], BF16, tag="tr", name="tr",
                                bufs=1)
                for j in range(DC):
                    nc.tensor.transpose(
                        tr[:, j, :], hb[:, j * 128:(j + 1) * 128], id_sb)
                sink(tr)

            def phase_A(b):
                # LN1 + quantize + PE-transpose into fp8 plane-blocked
                # [128, QD, 2, NLOC]
                hT8 = p_hT8.tile([128, QD, 2, NLOC], FP8, tag="hT8",
                                 name="hT8")
                hbs = []
                for i in range(TC):
                    xt = p_x.tile([128, D], F32, tag="x", name="x")
                    nc.sync.dma_start(
                        out=xt, in_=src.ap()[b, i * 128:(i + 1) * 128, :])
                    hbs.append(ln_norm(xt))

                    def sink(tr, i=i):
                        isl = slice(i * 128, (i + 1) * 128)
                        nc.vector.tensor_scalar_mul(
                            out=hT8[:, :, :, isl].rearrange(
                                "p c q e -> p (c q) e"),
                            in0=tr, scalar1=S_H)
                    if i > 0:
                        pe_transpose(hbs[i - 1], sinks.pop(0))
                    sinks.append(sink)
                pe_transpose(hbs[-1], sinks.pop(0))
                state["hT8"] = hT8

            def phase_B(b):
                hT8 = state["hT8"]
                k8 = p_k8.tile([128, TC, D], FP8, tag="k8", name="k8")
                v8 = p_v8.tile([128, TC, H, HD + 1], FP8, tag="v8",
                               name="v8")
                nc.vector.memset(v8[:, :, :, HD:HD + 1], 1.0)
                for i in range(TC):
                    isl = slice(i * 128, (i + 1) * 128)
                    for half in range(2):
                        csl = slice(half * 512, (half + 1) * 512)
                        # ---- k ----
                        pk = ps_mm.tile([128, 512], F32, tag="mm", name="pk")
                        for c in range(QD):
                            nc.tensor.matmul(
                                pk, hT8[:, c, :, isl], wk8s[c][:, :, csl],
                                start=(c == 0), stop=(c == QD - 1),
                                perf_mode=DRM)
                        if has_ckv:
                            kb = p_tmp.tile([128, 512], F32, tag="kb",
                                            name="kb")
                            nc.vector.scalar_tensor_tensor(
                                out=kb, in0=pk, scalar=cst_sb[:, C_SK_DQ:C_SK_DQ + 1],
                                in1=ck_b[:, csl], op0=ALU.mult, op1=ALU.add)
                            rl = p_ae.tile([128, 512], BF16, tag="ae",
                                           name="rl")
                            nc.scalar.activation(
                                out=rl, in_=kb, func=AF.Relu,
                                scale=cst_sb[:, C_SK_RL:C_SK_RL + 1])
                            # scale here must be S_K (kb is dequantized);
                            # C_SK_RL holds S_K when has_ckv (host-side).
                            mn = p_tmp.tile([128, 512], F32, tag="mn",
                                            name="mn")
                            nc.vector.tensor_scalar_min(out=mn, in0=kb,
                                                        scalar1=0.0)
                            ex = p_ae.tile([128, 512], BF16, tag="ae",
                                           name="ex")
                            nc.scalar.activation(
                                out=ex, in_=mn, func=AF.Exp,
                                bias=cst_sb[:, C_SK_LN:C_SK_LN + 1])
                            nc.gpsimd.tensor_add(out=k8[:, i, csl],
                                                 in0=rl, in1=ex)
                        else:
                            rl = p_ae.tile([128, 512], BF16, tag="ae",
                                           name="rl")
                            nc.vector.tensor_scalar(
                                out=rl, in0=pk,
                                scalar1=cst_sb[:, C_SK_RL:C_SK_RL + 1],
                                scalar2=0.0, op0=ALU.mult, op1=ALU.max)
                            # exp(min(k,0))*S == min(S*exp(k), S)
                            ex = p_ae.tile([128, 512], BF16, tag="ae",
                                           name="ex")
                            nc.scalar.activation(
                                out=ex, in_=pk, func=AF.Exp,
                                scale=cst_sb[:, C_SK_DQ:C_SK_DQ + 1],
                                bias=cst_sb[:, C_SK_LN:C_SK_LN + 1])
                            nc.vector.scalar_tensor_tensor(
                                out=k8[:, i, csl], in0=ex,
                                scalar=cst_sb[:, C_SK:C_SK + 1], in1=rl,
                                op0=ALU.min, op1=ALU.add)
                        # ---- v ----
                        pv = ps_mm.tile([128, 512], F32, tag="mm", name="pv")
                        for c in range(QD):
                            nc.tensor.matmul(
                                pv, hT8[:, c, :, isl], wv8s[c][:, :, csl],
                                start=(c == 0), stop=(c == QD - 1),
                                perf_mode=DRM)
                        vdst = v8[:, i, half * 8:(half + 1) * 8, 0:HD]
                        pvr = pv.rearrange("p (h e) -> p h e", e=HD)
                        if has_ckv:
                            cvr = cv_b[:, csl].rearrange(
                                "p (h e) -> p h e", e=HD)
                            nc.vector.scalar_tensor_tensor(
                                out=vdst, in0=pvr,
                                scalar=cst_sb[:, C_SV_M:C_SV_M + 1], in1=cvr,
                                op0=ALU.mult, op1=ALU.add)
                        else:
                            nc.vector.tensor_scalar_mul(
                                out=vdst, in0=pvr,
                                scalar1=cst_sb[:, C_SV_M:C_SV_M + 1])
                # kv accumulation over token chunk-pairs (DoubleRow)
                pkv = ps_kv.tile([128, NPAIR, 128], F32, tag="kv",
                                 name="pkv")
                for hp in range(NPAIR):
                    hA, hB = 2 * hp, 2 * hp + 1
                    for tp in range(TC // 2):
                        tsl = slice(2 * tp, 2 * tp + 2)
                        nc.tensor.matmul(
                            pkv[0:64, hp, 0:HD + 1],
                            k8[:, tsl, hA * HD:(hA + 1) * HD],
                            v8[:, tsl, hA, :],
                            start=(tp == 0), stop=(tp == TC // 2 - 1),
                            perf_mode=DRM,
                            tile_position=(0, 0), skip_group_check=True)
                    # DoubleRow cannot write at output-partition base 64
                    # (s3d3_mm_valid_dst_partition): single-rate for head B
                    for t in range(TC):
                        nc.tensor.matmul(
                            pkv[64:128, hp, 0:HD + 1],
                            k8[:, t, hB * HD:(hB + 1) * HD],
                            v8[:, t, hB, :],
                            start=(t == 0), stop=(t == TC - 1),
                            tile_position=(0, 64), skip_group_check=True)
                kv_sb = p_sm.tile([128, NPAIR, HD + 1], F32, tag="kvsb",
                                  name="kvsb")
                nc.vector.tensor_copy(out=kv_sb, in_=pkv[:, :, 0:HD + 1])
                kv_in = dram.tile([128, NPAIR, HD + 1], F32, tag="kvin",
                                  name="kvin")
                kv_out = dram_s.tile([128, NPAIR, HD + 1], F32, tag="kvout",
                                     name="kvout", addr_space="Shared")
                nc.sync.dma_start(out=kv_in, in_=kv_sb)
                nc.gpsimd.collective_compute(
                    "AllReduce", ALU.add,
                    replica_groups=[list(range(N_CORES))],
                    ins=[kv_in.opt()], outs=[kv_out.opt()])
                state["kv_out"] = kv_out

            def phase_B3(b):
                hT8 = state["hT8"]
                qT8 = p_qT8.tile([128, DC, NLOC], FP8, tag="qT8",
                                 name="qT8")
                for m in range(DC):
                    for th in range(2):
                        tsl = slice(th * 512, (th + 1) * 512)
                        pq = ps_mm.tile([128, 512], F32, tag="mm", name="pq")
                        for c in range(QD):
                            nc.tensor.matmul(
                                pq, wq8s[c][:, :, m * 128:(m + 1) * 128],
                                hT8[:, c, :, tsl],
                                start=(c == 0), stop=(c == QD - 1),
                                perf_mode=DRM)
                        if has_ckv:
                            qb = p_tmp.tile([128, 512], F32, tag="kb",
                                            name="qb")
                            nc.vector.tensor_scalar(
                                out=qb, in0=pq, scalar1=cdq_sb[:, m:m + 1],
                                scalar2=cqb_sb[:, m:m + 1],
                                op0=ALU.mult, op1=ALU.add)
                            rl = p_ae.tile([128, 512], BF16, tag="ae",
                                           name="rlq")
                            nc.scalar.activation(
                                out=rl, in_=qb, func=AF.Relu,
                                scale=cdqs_sb[:, m:m + 1])
                            # cdqs holds S_Q when has_ckv (host-side)
                            mn = p_tmp.tile([128, 512], F32, tag="mn",
                                            name="mnq")
                            nc.vector.tensor_scalar_min(out=mn, in0=qb,
                                                        scalar1=0.0)
                            ex = p_ae.tile([128, 512], BF16, tag="ae",
                                           name="exq")
                            nc.scalar.activation(
                                out=ex, in_=mn, func=AF.Exp,
                                bias=cst_sb[:, C_SQ_LN:C_SQ_LN + 1])
                            nc.gpsimd.tensor_add(out=qT8[:, m, tsl],
                                                 in0=rl, in1=ex)
                        else:
                            rl = p_ae.tile([128, 512], BF16, tag="ae",
                                           name="rlq")
                            nc.vector.tensor_scalar(
                                out=rl, in0=pq, scalar1=cdqs_sb[:, m:m + 1],
                                scalar2=0.0, op0=ALU.mult, op1=ALU.max)
                            ex = p_ae.tile([128, 512], BF16, tag="ae",
                                           name="exq")
                            nc.scalar.activation(
                                out=ex, in_=pq, func=AF.Exp,
                                scale=cdq_sb[:, m:m + 1],
                                bias=cst_sb[:, C_SQ_LN:C_SQ_LN + 1])
                            nc.vector.scalar_tensor_tensor(
                                out=qT8[:, m, tsl], in0=ex,
                                scalar=cst_sb[:, C_SQ:C_SQ + 1], in1=rl,
                                op0=ALU.min, op1=ALU.add)
                state["qT8"] = qT8

            def phase_D(b):
                qT8 = state["qT8"]
                kv_red = p_sm.tile([128, NPAIR, HD + 1], F32, tag="kvred",
                                   name="kvred")
                nc.sync.dma_start(out=kv_red, in_=state["kv_out"])
                # normalizer first (longest dependency chain): block-diag
                # ksum matmuls over all head pairs
                ks16 = p_sm.tile([128, 16], FP8, tag="ks16", name="ks16")
                nc.vector.memset(ks16, 0.0)
                nc.vector.tensor_scalar_mul(
                    out=ks16[0:64, :].rearrange("p (h two) -> p h two",
                                                two=2)[:, :, 0:1],
                    in0=kv_red[0:64, :, HD:HD + 1],
                    scalar1=cst_sb[0:64, C_KS_M:C_KS_M + 1])
                nc.vector.tensor_scalar_mul(
                    out=ks16[64:128, :].rearrange("p (h two) -> p h two",
                                                  two=2)[:, :, 1:2],
                    in0=kv_red[64:128, :, HD:HD + 1],
                    scalar1=cst_sb[64:128, C_KS_M:C_KS_M + 1])
                pn = [ps_mm.tile([16, 512], F32, tag="mm", name="pn")
                      for _ in range(2)]
                for hp in range(NPAIR):
                    for th in range(2):
                        nc.tensor.matmul(
                            pn[th], ks16,
                            qT8[:, hp, th * 512:(th + 1) * 512],
                            start=(hp == 0), stop=(hp == NPAIR - 1),
                            skip_group_check=True)
                n16 = p_sm.tile([16, NLOC], F32, tag="n16", name="n16")
                for th in range(2):
                    nc.vector.tensor_scalar(
                        out=n16[:, th * 512:(th + 1) * 512], in0=pn[th],
                        scalar1=cst_sb[0:16, C_N_M:C_N_M + 1],
                        scalar2=EPS_NORM, op0=ALU.mult, op1=ALU.add)
                nc.vector.reciprocal(out=n16, in_=n16)
                rn16 = p_sm.tile([16, NLOC], BF16, tag="rn16", name="rn16")
                with nc.allow_low_precision(reason="rn broadcast in bf16"):
                    nc.vector.tensor_scalar_mul(
                        out=rn16, in0=n16, scalar1=cst_sb[0:16, C_RN_M:C_RN_M + 1])
                rn_d = dram.tile([16, NLOC], BF16, tag="rnd", name="rnd")
                nc.sync.dma_start(out=rn_d, in_=rn16)

                kv8 = p_sm.tile([128, NPAIR, 128], FP8, tag="kv8",
                                name="kv8")
                nc.vector.memset(kv8, 0.0)
                nc.vector.tensor_scalar_mul(
                    out=kv8[0:64, :, 0:HD], in0=kv_red[0:64, :, 0:HD],
                    scalar1=cst_sb[0:64, C_KV_M:C_KV_M + 1])
                nc.vector.tensor_scalar_mul(
                    out=kv8[64:128, :, HD:128], in0=kv_red[64:128, :, 0:HD],
                    scalar1=cst_sb[64:128, C_KV_M:C_KV_M + 1])

                aT8 = p_aT8.tile([128, NPAIR, NLOC], FP8, tag="aT8",
                                 name="aT8")
                rnap = rn_d.opt()
                for hp in range(NPAIR):
                    for th in range(2):
                        csl = slice(th * 512, (th + 1) * 512)
                        rnb = p_rnb.tile([128, 512], BF16, tag="rnb",
                                         name="rnb", bufs=4)
                        for hh in range(2):
                            nc.sync.dma_start(
                                out=rnb[hh * 64:(hh + 1) * 64, :],
                                in_=bass.AP(
                                    tensor=rnap.tensor,
                                    offset=(rnap.offset
                                            + (2 * hp + hh) * NLOC
                                            + th * 512),
                                    ap=[[0, 64], [1, 512]]))
                        po = ps_mm.tile([128, 512], F32, tag="mm", name="po")
                        nc.tensor.matmul(
                            po, kv8[:, hp, :], qT8[:, hp, csl],
                            start=True, stop=True)
                        nc.vector.tensor_mul(
                            out=aT8[:, hp, csl], in0=po, in1=rnb)
                state["aT8"] = aT8

            def phase_E(b):
                aT8 = state["aT8"]
                s2d = [dram_s2.tile([128, D], F32, tag="s2d", name="s2d")
                       for _ in range(TC)]
                if fc1_fp8:
                    h2T8 = p_h2T8.tile([128, QD, 2, NLOC], FP8, tag="h2T8",
                                       name="h2T8")
                else:
                    h2T = p_h2T.tile([128, TC, DC, 128], BF16, tag="h2T",
                                     name="h2T")
                hbs2 = []
                for i in range(TC):
                    isl = slice(i * 128, (i + 1) * 128)
                    x2 = p_x.tile([128, D], F32, tag="x", name="x2")
                    nc.sync.dma_start(out=x2, in_=src.ap()[b, isl, :])
                    s2 = p_s2.tile([128, D], F32, tag="s2", name="s2")
                    for half in range(2):
                        csl = slice(half * 512, (half + 1) * 512)
                        pw = ps_mm.tile([128, 512], F32, tag="mm", name="pw")
                        for c in range(PP):
                            nc.tensor.matmul(
                                pw, aT8[:, 2 * c:2 * c + 2, isl],
                                wo8s[c][:, :, csl],
                                start=(c == 0), stop=(c == PP - 1),
                                perf_mode=DRM)
                        nc.vector.scalar_tensor_tensor(
                            out=s2[:, csl], in0=pw,
                            scalar=cst_sb[:, C_WO_DQ:C_WO_DQ + 1],
                            in1=x2[:, csl],
                            op0=ALU.mult, op1=ALU.add)
                    nc.sync.dma_start(out=s2d[i], in_=s2)
                    hbs2.append(ln_norm(s2))

                    if fc1_fp8:
                        def sink(tr, i=i):
                            isl2 = slice(i * 128, (i + 1) * 128)
                            nc.vector.tensor_scalar_mul(
                                out=h2T8[:, :, :, isl2].rearrange(
                                    "p c q e -> p (c q) e"),
                                in0=tr, scalar1=S_H)
                    else:
                        def sink(tr, i=i):
                            nc.vector.tensor_copy(
                                out=h2T[:, i, :, :], in_=tr)
                    if i > 0:
                        pe_transpose(hbs2[i - 1], sinks.pop(0))
                    sinks.append(sink)
                pe_transpose(hbs2[-1], sinks.pop(0))
                if fc1_fp8:
                    state["h2T8"] = h2T8
                else:
                    state["h2T"] = h2T
                state["s2d"] = s2d

            def phase_GH(b, s2d, h2x, th):
                    tsl = slice(th * 512, (th + 1) * 512)
                    gts = [p_gt.tile([128, 512], BF16, tag="gt", name="gt")
                           for _ in range(GC)]
                    for m in range(GC):
                        pu = ps_mm.tile([128, 512], F32, tag="mm", name="pu")
                        if fc1_fp8:
                            for c in range(QD):
                                nc.tensor.matmul(
                                    pu,
                                    f18s[c][:, :, m * 128:(m + 1) * 128],
                                    h2x[:, c, :, tsl],
                                    start=(c == 0), stop=(c == QD - 1),
                                    perf_mode=DRM)
                            nc.scalar.activation(
                                out=gts[m], in_=pu, func=AF.Gelu,
                                bias=c1b_sb[:, m:m + 1],
                                scale=cdg_sb[:, m:m + 1])
                        else:
                            f1t = p_f1.tile([128, DC, 128], BF16, tag="f1",
                                            name="f1t")
                            nc.sync.dma_start(
                                out=f1t,
                                in_=fc1.ap()[m].rearrange(
                                    "p (j e) -> p j e", j=DC))
                            for j in range(DC):
                                nc.tensor.matmul(
                                    pu, f1t[:, j, :],
                                    h2x[:, 4 * th:4 * th + 4, j, :],
                                    start=(j == 0), stop=(j == DC - 1))
                            nc.scalar.activation(
                                out=gts[m], in_=pu, func=AF.Gelu,
                                bias=c1b_sb[:, m:m + 1], scale=1.0)
                    for ncol in range(2):
                        csl = slice(ncol * 512, (ncol + 1) * 512)
                        py2 = [ps_mm.tile([128, 512], F32, tag="mm",
                                          name="py2") for _ in range(4)]
                        for m in range(GC):
                            f2t = p_f2.tile([128, 512], BF16, tag="f2",
                                            name="f2t")
                            nc.sync.dma_start(out=f2t,
                                              in_=fc2.ap()[m][:, csl])
                            for ii in range(4):
                                nc.tensor.matmul(
                                    py2[ii],
                                    gts[m][:, ii * 128:(ii + 1) * 128],
                                    f2t,
                                    start=(m == 0), stop=(m == GC - 1))
                        for ii in range(4):
                            i = th * 4 + ii
                            s2c = p_ob.tile([128, 512], F32, tag="s2c",
                                            name="s2c")
                            nc.sync.dma_start(out=s2c, in_=s2d[i][:, csl])
                            ot = p_ob.tile([128, 512], F32, tag="ot",
                                           name="ot")
                            if has_c2:
                                nc.vector.scalar_tensor_tensor(
                                    out=ot, in0=py2[ii], scalar=0.0,
                                    in1=c2_b[:, csl], op0=ALU.add,
                                    op1=ALU.add)
                                nc.vector.tensor_add(out=ot, in0=ot,
                                                     in1=s2c)
                            else:
                                nc.vector.tensor_add(out=ot, in0=py2[ii],
                                                     in1=s2c)
                            nc.sync.dma_start(
                                out=out.ap()[b, i * 128:(i + 1) * 128, csl],
                                in_=ot)

            # ---------------- pipelined emission --------------------------
            # per iteration: B(b) B3(b) GH1(b-1) D(b) E(b) GH0(b) A(b+1)
            # GH1(b-1) covers the AllReduce latency of batch b; GH0(b)
            # covers the LN/transpose work of batch b+1.
            phase_A(0)
            prev = None
            for b in range(B):
                if b > 0:
                    phase_B(b)
                    phase_B3(b)
                else:
                    # batch 0 has no prior-MLP work to cover its AllReduce:
                    # pull batch 1's LN phase forward instead
                    phase_B(0)
                    phase_B3(0)
                    phase_A(1)
                if prev is not None:
                    phase_GH(*prev, 1)
                phase_D(b)
                phase_E(b)
                cur = (b, state["s2d"],
                       state["h2T8"] if fc1_fp8 else state["h2T"])
                phase_GH(*cur, 0)
                if b + 1 < B and b > 0:
                    phase_A(b + 1)
                prev = cur
            phase_GH(*prev, 1)

    _nc_cache[key] = nc
    return nc


def _p2(x, target):
    """Largest power of 2 <= target / x."""
    x = float(max(x, 1e-30))
    return 2.0 ** math.floor(math.log2(target / x))


def _phi(x):
    return np.where(x > 0, x + 1.0, np.exp(np.minimum(x, 0.0)))


def _q8(x, s):
    return np.clip(x * s, -240.0, 240.0).astype(E4)


def prepare(inputs):
    """Host-side packing: returns (nc, in_maps)."""
    src = np.ascontiguousarray(np.asarray(inputs["src"], dtype=np.float32))
    ln1_w = np.asarray(inputs["ln1_w"], np.float32)
    ln1_b = np.asarray(inputs["ln1_b"], np.float32)
    wq = np.asarray(inputs["wq"], np.float32)
    wk = np.asarray(inputs["wk"], np.float32)
    wv = np.asarray(inputs["wv"], np.float32)
    wo = np.asarray(inputs["wo"], np.float32)
    ln2_w = np.asarray(inputs["ln2_w"], np.float32)
    ln2_b = np.asarray(inputs["ln2_b"], np.float32)
    fc1_w = np.asarray(inputs["fc1_w"], np.float32)
    fc1_b = np.asarray(inputs["fc1_b"], np.float32)
    fc2_w = np.asarray(inputs["fc2_w"], np.float32)
    fc2_b = np.asarray(inputs["fc2_b"], np.float32)

    wqf = ln1_w[:, None] * wq
    wkf = ln1_w[:, None] * wk
    wvf = ln1_w[:, None] * wv
    fc1f = ln2_w[:, None] * fc1_w
    cq_v = ln1_b @ wq
    ck_v = ln1_b @ wk
    cv_v = ln1_b @ wv
    c1_v = ln2_b @ fc1_w + fc1_b
    has_ckv = bool(np.any(ck_v) or np.any(cv_v) or np.any(cq_v))
    has_c2 = bool(np.any(fc2_b))

    # ---- scale estimation on a token subsample ----
    xs = src[:, ::16, :].reshape(-1, D)          # 2048 tokens
    mu = xs.mean(-1, keepdims=True)
    var = xs.var(-1)
    hn = (xs - mu) / np.sqrt(var + EPS_LN)[:, None]
    qs = hn @ wqf + cq_v
    ks = hn @ wkf + ck_v
    vs = hn @ wvf + cv_v
    pq, pk = _phi(qs), _phi(ks)
    S_K = _p2(np.abs(pk).max() * 1.3, 120.0)
    S_Q = _p2(np.abs(pq).max() * 1.3, 120.0)
    S_V = _p2(np.abs(vs).max() * 1.3, 120.0)
    nsamp = xs.shape[0] // B
    scale_n = N / nsamp
    pkb = pk.reshape(B, nsamp, H, HD)
    vsb = vs.reshape(B, nsamp, H, HD)
    kv_est = np.einsum("bshd,bshe->bhde", pkb, vsb) * scale_n
    ks_est = pkb.sum(1) * scale_n
    S_KV = _p2(np.abs(kv_est).max() * 1.3, 110.0)
    S_KS = _p2(np.abs(ks_est).max() * 1.3, 110.0)
    n_est = np.einsum("bshd,bhd->bsh", pq.reshape(B, nsamp, H, HD),
                      ks_est) + EPS_NORM
    a_est = np.einsum("bshd,bhde->bshe", pq.reshape(B, nsamp, H, HD),
                      kv_est) / n_est[..., None]
    S_A = _p2(np.abs(a_est).max() * 1.5, 120.0)
    S_WK = _p2(np.abs(wkf).max(), 120.0)
    S_WV = _p2(np.abs(wvf).max(), 120.0)
    S_WO = _p2(np.abs(wo).max(), 120.0)
    S_WQ = np.array([_p2(a, 120.0) for a in
                     np.abs(wqf).max(0)], np.float32)       # per out column
    S_H2 = S_H

    # ---- weight packing ----
    def pack_planes(w, s):
        # [QD, 128, 2, F]: w8[c, p, q, f] = Q(w[128*(2c+q)+p, f] * s)
        F = w.shape[1]
        return np.ascontiguousarray(
            _q8(w, s).reshape(QD, 2, 128, F).transpose(0, 2, 1, 3))

    base = {
        "ident": np.eye(128, dtype=BF),
        "wk8": pack_planes(wkf, S_WK),
        "wv8": pack_planes(wvf, S_WV),
        "wq8": pack_planes(wqf, S_WQ[None, :]),
        "wo8": pack_planes(wo, S_WO),
        "fc2": fc2_w.astype(BF).reshape(GC, 128, D),
    }
    if FC1_FP8:
        S_F1 = np.array([_p2(a, 120.0) for a in np.abs(fc1f).max(0)],
                        np.float32)
        base["fc18"] = pack_planes(fc1f, S_F1[None, :])
        base["cdg"] = np.broadcast_to(
            (1.0 / (S_H2 * S_F1)).reshape(GC, 128).T,
            (128, GC)).copy().astype(np.float32)
        # note: reshape(GC,128).T maps [p, m] -> S_F1[m*128+p]
    else:
        base["fc1"] = np.ascontiguousarray(
            fc1f.astype(BF).reshape(DC, 128, GC, 128)
            .transpose(2, 1, 0, 3).reshape(GC, 128, D))

    cst_v = np.zeros(NCST, np.float32)
    if has_ckv:
        cst_v[C_SK_RL] = S_K
    else:
        cst_v[C_SK_RL] = S_K / (S_H * S_WK)
    cst_v[C_SK_DQ] = 1.0 / (S_H * S_WK)
    cst_v[C_SK_LN] = math.log(S_K)
    cst_v[C_SV_M] = S_V / (S_H * S_WV)
    cst_v[C_KV_M] = S_KV / (S_K * S_V)
    cst_v[C_KS_M] = S_KS / S_K
    cst_v[C_N_M] = 1.0 / (S_KS * S_Q)
    cst_v[C_RN_M] = S_A / (S_KV * S_Q)
    cst_v[C_WO_DQ] = 1.0 / (S_A * S_WO)
    cst_v[C_SQ_LN] = math.log(S_Q)
    cst_v[C_SK] = S_K
    cst_v[C_SQ] = S_Q
    base["cst"] = np.broadcast_to(cst_v, (128, NCST)).copy()
    if has_ckv:
        base["cdqs"] = np.broadcast_to(
            np.float32(S_Q), (128, DC)).copy()
    else:
        base["cdqs"] = np.ascontiguousarray(
            (S_Q / (S_H * S_WQ)).reshape(DC, 128).T.astype(np.float32))
    base["cdq"] = np.ascontiguousarray(
        (1.0 / (S_H * S_WQ)).reshape(DC, 128).T.astype(np.float32))
    base["c1b"] = np.ascontiguousarray(
        c1_v.reshape(GC, 128).T.astype(np.float32))
    if has_ckv:
        base["ckv"] = np.stack([ck_v, S_V * cv_v]).astype(np.float32)
        base["cqb2"] = np.concatenate([
            np.ascontiguousarray(cq_v.reshape(DC, 128).T),
            np.zeros((128, DC), np.float32)], axis=1).astype(np.float32)
    if has_c2:
        base["c2"] = fc2_b.astype(np.float32)

    nc = _build(has_ckv, has_c2, FC1_FP8)
    in_maps = []
    for c in range(N_CORES):
        m = dict(base)
        m["src"] = np.ascontiguousarray(src[:, c * NLOC:(c + 1) * NLOC, :])
        in_maps.append(m)
    return nc, in_maps


def kernel(**inputs) -> np.ndarray:
    nc, in_maps = prepare(inputs)
    res = bass_utils.run_bass_kernel_spmd(
        nc, in_maps, core_ids=list(range(N_CORES)))
    return np.concatenate(
        [res.results[c]["out"] for c in range(N_CORES)], axis=1)
